# revision 10
# baseline (speedup 1.0000x reference)
# Transformer-XL style relative-position attention on 8 Trainium2 NeuronCores.
#
# Contract: kernel(**inputs) takes the FULL unsharded inputs and returns the
# FULL [8, 256, 1024] output. Internally shards data-parallel over batch:
# core b computes batch element b. No collectives needed.
#
# Math (per batch element):
#   cat = [h; x]                            [512, 1024]
#   q,k,v = split(cat @ Wqkv)               heads=16, dhead=64
#   RW    = R @ Wkr                         [1024, 1024] (relative pos keys)
#   dots  = (q+u) @ k^T + rel_shift((q+v) @ RW_h^T)
#   out   = softmax(dots*8^-1 + causal/mem band mask) @ v @ Wout
#
# Key facts exploited:
#  * The combined mem/autoregressive mask keeps exactly the relative offsets
#    j - i in [0, 256]; in rel-coordinate s = j - i + 256 the valid window is
#    s in [256, 512] (257 values), so only 257 rows of RW are ever needed
#    (R rows 768..1023 and 0, since RW row (s+512)%1024 serves offset s).
#  * rel_shift is a per-row shear. SBUF cannot be addressed diagonally, but
#    DRAM can: write the [128, 257] valid band of BDs = (q+v) @ RWs^T to a
#    DRAM scratch laid out [128, 767] and read it back with the access
#    pattern [[766, 128], [1, 512]] (row stride 767-1) which realizes
#    band[i, j] = BDs[i, j - i + const]. The read is DMA-accumulated onto
#    the (q+u)k^T + mask tile, the masked-out region of the scratch is
#    zeroed once so the accumulate is a no-op there.
#  * Normalization 1/S is applied to exp rows (per-partition scalar) before
#    the PE transpose into key-major layout used by the AV matmul.
#  * All large matmuls use dtype float32r (full fp32 storage, fast PE path).

import numpy as np

import concourse.bass as bass
import concourse.mybir as mybir
import concourse.tile as tile
from concourse import bacc, bass_utils
from concourse.masks import make_identity
from concourse.tile import add_dep_helper
from contextlib import ExitStack

F32 = mybir.dt.float32
F32R = mybir.dt.float32r
BF16 = mybir.dt.float16  # fp16: 10-bit mantissa, 1cyc/row like bf16
AF = mybir.ActivationFunctionType

DIM = 1024
HEADS = 16
DHEAD = 64
B = 8
N = 256          # query tokens (x)
M = 256          # memory tokens (h)
T = M + N        # 512 keys
INNER = HEADS * DHEAD
SCALE = DHEAD ** -0.5
NEG = -1.0e9
SW = 767         # BDs scratch width (relative offsets s = 1..767)
VAL0 = 255       # scratch col of first valid offset (s = 256)
NVALID = 257     # valid offsets s in [256, 512]
NV2 = 258        # NVALID padded even (fp32r matmul dst requires even width)
NBUF = 4         # BDs scratch double-buffering depth

MM_DT = F32R     # dtype for the heavy matmuls


def _mm(ap):
    return ap


def build_kernel():
    nc = bacc.Bacc("TRN2", target_bir_lowering=False, debug=False)

    x_d = nc.dram_tensor("x", [N, DIM], F32, kind="ExternalInput")
    h_d = nc.dram_tensor("h", [M, DIM], F32, kind="ExternalInput")
    wqkv_d = nc.dram_tensor("Wqkv", [DIM, 3 * INNER], F32, kind="ExternalInput")
    wkr_d = nc.dram_tensor("Wkr", [DIM, INNER], F32, kind="ExternalInput")
    r_d = nc.dram_tensor("R", [2 * T, DIM], F32, kind="ExternalInput")
    uu_d = nc.dram_tensor("uu", [128, 1], F32, kind="ExternalInput")
    vv_d = nc.dram_tensor("vv", [128, 1], F32, kind="ExternalInput")
    wout_d = nc.dram_tensor("Wout", [INNER, DIM], F32, kind="ExternalInput")
    out_d = nc.dram_tensor("out", [N, DIM], F32, kind="ExternalOutput")
    bds_d = nc.dram_tensor("bds_scratch", [NBUF, 128, SW], F32)

    with tile.TileContext(nc) as tc, ExitStack() as ctx:
        _body(ctx, tc, x_d, h_d, wqkv_d, wkr_d, r_d, uu_d, vv_d, wout_d,
              out_d, bds_d)

    nc.compile()
    return nc


def _body(ctx, tc, x_d, h_d, wqkv_d, wkr_d, r_d, uu_d, vv_d, wout_d, out_d,
          bds_d):
    nc = tc.nc

    const = ctx.enter_context(tc.tile_pool(name="const", bufs=1))
    persist = ctx.enter_context(tc.tile_pool(name="persist", bufs=1))
    ldpool = ctx.enter_context(tc.tile_pool(name="ld", bufs=4))
    wpool = ctx.enter_context(tc.tile_pool(name="wstream", bufs=3))
    work = ctx.enter_context(tc.tile_pool(name="work", bufs=4))
    ps_big = ctx.enter_context(tc.tile_pool(name="ps_big", bufs=2, space="PSUM"))
    ps_mid = ctx.enter_context(tc.tile_pool(name="ps_mid", bufs=2, space="PSUM"))
    ps_sml = ctx.enter_context(tc.tile_pool(name="ps_sml", bufs=2, space="PSUM"))

    # ---------------- constants ----------------
    ident = const.tile([128, 128], F32, tag="ident", name="ident")
    make_identity(nc, ident)

    # bf16 identity for the attention-probability transposes
    ident_bf = const.tile([128, 128], BF16, tag="identb", name="ident_bf")
    make_identity(nc, ident_bf)

    # The scratch is pre-filled with the additive mask value: every column
    # outside the per-iteration [VAL0, VAL0+NV2) band write stays NEG, and
    # the band write itself puts NEG in its pad column. The band read then
    # delivers band+mask in one tensor.
    neg_sb = const.tile([128, SW], F32, tag="zero", name="neg_sb")
    nc.gpsimd.memset(neg_sb, NEG)

    uu = const.tile([128, 1], F32, tag="uu", name="uu_sb")
    vv = const.tile([128, 1], F32, tag="vv", name="vv_sb")
    nc.sync.dma_start(out=uu, in_=uu_d[:, :])
    nc.sync.dma_start(out=vv, in_=vv_d[:, :])

    # zero-init the BDs scratch (regions outside the valid band stay 0)
    zinit = []
    for bi in range(NBUF):
        zi = nc.sync.dma_start(out=bds_d[bi], in_=neg_sb)
        zinit.append(zi)

    # ---------------- load + transpose x, h, R ----------------
    # cat token order: [h (0:256) | x (256:512)]
    cat_nat = []
    for tt in range(4):
        t_ = ldpool.tile([128, DIM], F32, tag="xh", name=f"cat_nat{tt}")
        src = h_d if tt < 2 else x_d
        nc.sync.dma_start(out=t_, in_=src[(tt % 2) * 128:(tt % 2) * 128 + 128, :])
        cat_nat.append(t_)

    catT = [persist.tile([128, T], F32R, tag=f"catT{dt}", name=f"catT{dt}")
            for dt in range(8)]
    for tt in range(4):
        for dt in range(8):
            tp = ps_sml.tile([128, 128], F32, tag="tp", name=f"tp_cat{tt}_{dt}")
            nc.tensor.transpose(tp, cat_nat[tt][:, dt * 128:(dt + 1) * 128], ident)
            nc.vector.tensor_copy(catT[dt][:, tt * 128:(tt + 1) * 128], tp)

    # R rows needed: offsets s=256..511 -> rows 768..1023; s=512 -> row 0
    r_nat = []
    for rt in range(2):
        t_ = ldpool.tile([128, DIM], F32, tag="rn", name=f"r_nat{rt}", bufs=2)
        nc.sync.dma_start(out=t_, in_=r_d[768 + rt * 128:768 + (rt + 1) * 128, :])
        r_nat.append(t_)
    r0 = const.tile([2, DIM], F32, tag="r0", name="r0_sb")
    nc.gpsimd.memset(r0, 0.0)
    nc.sync.dma_start(out=r0[0:1, :], in_=r_d[0:1, :])

    rsubT = [persist.tile([128, NV2], F32R, tag=f"rsubT{dt}", name=f"rsubT{dt}")
             for dt in range(8)]
    for rt in range(2):
        for dt in range(8):
            tp = ps_sml.tile([128, 128], F32, tag="tp", name=f"tp_r{rt}_{dt}")
            nc.tensor.transpose(tp, r_nat[rt][:, dt * 128:(dt + 1) * 128], ident)
            nc.vector.tensor_copy(rsubT[dt][:, rt * 128:(rt + 1) * 128], tp)
    for dt in range(8):
        tp = ps_sml.tile([128, 2], F32, tag="tp", name=f"tp_r0_{dt}")
        nc.tensor.transpose(tp, r0[:, dt * 128:(dt + 1) * 128], ident[0:2, 0:2])
        nc.vector.tensor_copy(rsubT[dt][:, 256:258], tp)

    # ---------------- projections ----------------
    def colslice_load(w_dram, col0, ft, name):
        """Load [1024, 128] column slice of a [1024, >=1024] DRAM tensor into
        a [128, 1024] SBUF tile laid out (p=dim%128, dt*128+c)."""
        wt = wpool.tile([128, DIM], F32R, tag="w1024", name=name)
        src = w_dram[:, col0 + ft * 128:col0 + (ft + 1) * 128].bitcast(F32R).rearrange(
            "(a p) c -> p a c", p=128)
        dst = wt.rearrange("p (a c) -> p a c", c=128)
        nc.sync.dma_start(out=dst, in_=src)
        return wt

    # k_T[ft] = [128 feat, 512 tok]
    kT = [persist.tile([128, T], F32R, tag=f"kT{ft}", name=f"kT{ft}")
          for ft in range(8)]
    for ft in range(8):
        wk = colslice_load(wqkv_d, INNER, ft, f"wk{ft}")
        pk = ps_mid.tile([128, T], F32, tag="mid", name=f"ps_k{ft}")
        for dt in range(8):
            nc.tensor.matmul(pk, _mm(wk[:, dt * 128:(dt + 1) * 128]),
                             _mm(catT[dt]), start=(dt == 0), stop=(dt == 7))
        nc.vector.tensor_copy(kT[ft], pk)

    # q_T (x tokens only) -> qu_T, qv_T [128 feat, 256 tok]
    quT = [persist.tile([128, N], F32R, tag=f"quT{ft}", name=f"quT{ft}")
           for ft in range(8)]
    qvT = [persist.tile([128, N], F32R, tag=f"qvT{ft}", name=f"qvT{ft}")
           for ft in range(8)]
    for ft in range(8):
        wq = colslice_load(wqkv_d, 0, ft, f"wq{ft}")
        pq = ps_mid.tile([128, N], F32, tag="mid", name=f"ps_q{ft}")
        for dt in range(8):
            nc.tensor.matmul(pq, _mm(wq[:, dt * 128:(dt + 1) * 128]),
                             _mm(catT[dt][:, M:T]), start=(dt == 0), stop=(dt == 7))
        nc.vector.tensor_scalar_add(quT[ft], pq, uu)
        nc.vector.tensor_scalar_add(qvT[ft], pq, vv)

    # RWs_T[ft] = [128 feat, 257 offsets]
    rwsT = [persist.tile([128, NV2], F32R, tag=f"rwsT{ft}", name=f"rwsT{ft}")
            for ft in range(8)]
    for ft in range(8):
        wr = colslice_load(wkr_d, 0, ft, f"wr{ft}")
        pr = ps_mid.tile([128, NV2], F32, tag="mid", name=f"ps_rw{ft}")
        for dt in range(8):
            nc.tensor.matmul(pr, _mm(wr[:, dt * 128:(dt + 1) * 128]),
                             _mm(rsubT[dt]), start=(dt == 0), stop=(dt == 7))
        nc.vector.tensor_copy(rwsT[ft], pr)

    # val[tt] = [128 tok, 1024 feat], two passes of two token tiles each
    val = [persist.tile([128, INNER], BF16, tag=f"val{tt}", name=f"val{tt}")
           for tt in range(4)]
    for half in range(2):
        pv = [ps_big.tile([128, INNER], F32, tag="big", name=f"ps_v{half}_{k}")
              for k in range(2)]
        for dt in range(8):
            wv = wpool.tile([128, INNER], F32R, tag="w1024", name=f"wv{half}_{dt}")
            nc.sync.dma_start(out=wv,
                              in_=wqkv_d[dt * 128:(dt + 1) * 128,
                                         2 * INNER:3 * INNER].bitcast(F32R))
            for k in range(2):
                tt = half * 2 + k
                lhs = catT[dt][:, tt * 128:(tt + 1) * 128]
                for nh in range(2):
                    nc.tensor.matmul(pv[k][:, nh * 512:(nh + 1) * 512],
                                     _mm(lhs), _mm(wv[:, nh * 512:(nh + 1) * 512]),
                                     start=(dt == 0), stop=(dt == 7))
        for k in range(2):
            nc.vector.tensor_copy(val[half * 2 + k], pv[k])

    # ---------------- attention ----------------
    attn_outT = [persist.tile([128, N], F32R, tag=f"aoT{ft}", name=f"aoT{ft}")
                 for ft in range(8)]

    last_read = [None] * NBUF   # WAR chain on the DRAM scratch
    it = 0
    for hh in range(HEADS):
        ft, ro = hh // 2, (hh % 2) * 64
        attnT = [work.tile([128, N], BF16, tag="attnT", name=f"attnT{hh}_{jt}",
                           bufs=8) for jt in range(4)]
        for qb in range(2):
            bi = it % NBUF
            qsl = slice(qb * 128, (qb + 1) * 128)

            # BDs = (q+v) @ RWs^T  -> valid band to DRAM scratch
            pb = ps_mid.tile([128, NV2], F32, tag="mid", name=f"ps_b{it}")
            nc.tensor.matmul(pb, _mm(qvT[ft][ro:ro + 64, qsl]),
                             _mm(rwsT[ft][ro:ro + 64, :]), start=True, stop=True)
            bsb = work.tile([128, NV2], F32, tag="bsb", name=f"bsb{it}")
            nc.vector.tensor_copy(bsb[:, 0:NVALID], pb[:, 0:NVALID])
            nc.vector.memset(bsb[:, NVALID:NV2], NEG)
            w_inst = nc.sync.dma_start(
                out=bds_d[bi][:, VAL0:VAL0 + NV2], in_=bsb)
            add_dep_helper(w_inst.ins, zinit[bi].ins, sync=True,
                           reason="scratch WAW zero-init")
            if last_read[bi] is not None:
                add_dep_helper(w_inst.ins, last_read[bi].ins, sync=True,
                               reason="scratch WAR reuse")

            # A = (q+u) @ k^T, then dots = A + mask (+ band via DMA-accum)
            pa = ps_mid.tile([128, T], F32, tag="mid", name=f"ps_a{it}")
            nc.tensor.matmul(pa, _mm(quT[ft][ro:ro + 64, qsl]),
                             _mm(kT[ft][ro:ro + 64, :]), start=True, stop=True)
            band_sb = work.tile([128, T], F32, tag="band", name=f"band{it}")
            band = bass.AP(bds_d[bi].tensor,
                           bi * 128 * SW + VAL0 - qb * 128,
                           [[SW - 1, 128], [1, T]])
            r_inst = nc.scalar.dma_start(out=band_sb, in_=band)
            add_dep_helper(r_inst.ins, w_inst.ins, sync=True,
                           reason="band RAW on scratch")
            add_dep_helper(r_inst.ins, zinit[bi].ins, sync=True,
                           reason="band RAW on mask-init")
            last_read[bi] = r_inst
            dots = work.tile([128, T], F32, tag="dots", name=f"dots{it}")
            nc.vector.tensor_add(dots, pa, band_sb)

            # exp(+row sums), normalize
            expt = work.tile([128, T], F32, tag="expt", name=f"expt{it}")
            ssum = work.tile([128, 1], F32, tag="ssum", name=f"ssum{it}", bufs=4)
            nc.scalar.activation(expt, dots, AF.Exp, bias=0.0, scale=SCALE,
                                 accum_out=ssum)
            rcp = work.tile([128, 1], F32, tag="rcp", name=f"rcp{it}", bufs=4)
            nc.vector.reciprocal(rcp, ssum)
            expn = work.tile([128, T], BF16, tag="expn", name=f"expn{it}")
            nc.vector.tensor_scalar_mul(expn, expt, rcp)

            # transpose attn rows into key-major attnT tiles
            for jt in range(4):
                tp = ps_sml.tile([128, 128], BF16, tag="tp", name=f"tp_e{it}_{jt}")
                nc.tensor.transpose(tp, expn[:, jt * 128:(jt + 1) * 128], ident_bf)
                nc.vector.tensor_copy(attnT[jt][:, qsl], tp)
            it += 1

        # out_T[h] = val^T-contract: [64 d, 256 i]
        pav = ps_sml.tile([64, N], F32, tag="tp", name=f"ps_av{hh}")
        for jt in range(4):
            nc.tensor.matmul(pav, _mm(val[jt][:, hh * 64:hh * 64 + 64]),
                             _mm(attnT[jt]), start=(jt == 0), stop=(jt == 3))
        nc.vector.tensor_copy(attn_outT[ft][ro:ro + 64, :], pav)

    # ---------------- output projection ----------------
    pp = [ps_big.tile([128, DIM], F32, tag="big", name=f"ps_o{tt}")
          for tt in range(2)]
    for itile in range(8):
        wo = wpool.tile([128, DIM], F32R, tag="w1024", name=f"wo{itile}")
        nc.sync.dma_start(out=wo, in_=wout_d[itile * 128:(itile + 1) * 128, :].bitcast(F32R))
        for tt in range(2):
            lhs = attn_outT[itile][:, tt * 128:(tt + 1) * 128]
            for nh in range(2):
                nc.tensor.matmul(pp[tt][:, nh * 512:(nh + 1) * 512],
                                 _mm(lhs), _mm(wo[:, nh * 512:(nh + 1) * 512]),
                                 start=(itile == 0), stop=(itile == 7))
    for tt in range(2):
        osb = work.tile([128, DIM], F32, tag="osb", name=f"osb{tt}", bufs=2)
        nc.scalar.copy(osb, pp[tt])
        nc.sync.dma_start(out=out_d[tt * 128:(tt + 1) * 128, :], in_=osb)


_NC_CACHE = {}


def _get_nc():
    if "nc" not in _NC_CACHE:
        _NC_CACHE["nc"] = build_kernel()
    return _NC_CACHE["nc"]


def _run(inputs, trace=False):
    x = np.ascontiguousarray(np.asarray(inputs["x"], dtype=np.float32))
    h = np.ascontiguousarray(np.asarray(inputs["h"], dtype=np.float32))
    wqkv = np.ascontiguousarray(np.asarray(inputs["Wqkv"], dtype=np.float32))
    wkr = np.ascontiguousarray(np.asarray(inputs["Wkr"], dtype=np.float32))
    r = np.ascontiguousarray(np.asarray(inputs["R"], dtype=np.float32))
    u = np.asarray(inputs["u"], dtype=np.float32)
    v = np.asarray(inputs["v"], dtype=np.float32)
    wout = np.ascontiguousarray(np.asarray(inputs["Wout"], dtype=np.float32))
    uu = np.ascontiguousarray(np.tile(u, 2).reshape(128, 1))
    vv = np.ascontiguousarray(np.tile(v, 2).reshape(128, 1))

    nc = _get_nc()
    in_maps = [
        {"x": x[b], "h": h[b], "Wqkv": wqkv, "Wkr": wkr, "R": r,
         "uu": uu, "vv": vv, "Wout": wout}
        for b in range(B)
    ]
    res = bass_utils.run_bass_kernel_spmd(
        nc, in_maps, core_ids=list(range(B)), trace=trace)
    out = np.stack([res.results[b]["out"] for b in range(B)])
    return out.astype(np.float32), res


def kernel(**inputs):
    out, _ = _run(inputs, trace=False)
    return out


# revision 12
# speedup vs baseline: 1.0851x; 1.0851x over previous
# Transformer-XL style relative-position attention on 8 Trainium2 NeuronCores.
#
# Contract: kernel(**inputs) takes the FULL unsharded inputs and returns the
# FULL [8, 256, 1024] output. Internally shards data-parallel over batch:
# core b computes batch element b. No collectives needed.
#
# Math (per batch element):
#   cat = [h; x]                            [512, 1024]
#   q,k,v = split(cat @ Wqkv)               heads=16, dhead=64
#   RW    = R @ Wkr                         [1024, 1024] (relative pos keys)
#   dots  = (q+u) @ k^T + rel_shift((q+v) @ RW_h^T)
#   out   = softmax(dots*8^-1 + causal/mem band mask) @ v @ Wout
#
# Key design points:
#  * The combined mem/autoregressive mask keeps exactly the relative offsets
#    j - i in [0, 256]; in rel-coordinate s = j - i + 256 the valid window is
#    s in [256, 512] (257 values), so only 257 rows of RW are ever needed
#    (R rows 768..1023 and 0, since RW row (s+512)%1024 serves offset s).
#  * rel_shift is a per-row shear. SBUF cannot be addressed diagonally, but
#    DRAM can: write the [128, 258] valid band of BDs = (q+v) @ RWs^T to a
#    DRAM scratch laid out [128, 767] and read it back with the access
#    pattern [[766, 128], [1, 512]] (row stride 767-1) which realizes
#    band[i, j] = BDs[i, j - i + const]. The scratch is pre-filled with the
#    additive mask value -1e9 and the band write puts -1e9 in its pad
#    column, so the band read delivers band+mask in a single tensor.
#  * All matmul operands are fp16 (halves LDWEIGHTS streaming, which
#    dominates the PE pipe); accumulation stays fp32 in PSUM, and the
#    softmax/logit path (dots, exp, row sums) stays fp32.
#  * Weights are cast f32->f16 in-flight by gpsimd (SWDGE) cast-DMAs: no
#    compute-engine time and fully contiguous row reads.
#  * Normalization 1/S is applied per-partition to exp rows before the PE
#    transpose into the key-major layout used by the AV matmul.

import numpy as np

import concourse.bass as bass
import concourse.mybir as mybir
import concourse.tile as tile
from concourse import bacc, bass_utils
from concourse.masks import make_identity
from concourse.tile import add_dep_helper
from contextlib import ExitStack

F32 = mybir.dt.float32
F16 = mybir.dt.float16
AF = mybir.ActivationFunctionType

DIM = 1024
HEADS = 16
DHEAD = 64
B = 8
N = 256          # query tokens (x)
M = 256          # memory tokens (h)
T = M + N        # 512 keys
INNER = HEADS * DHEAD
SCALE = DHEAD ** -0.5
NEG = -1.0e9
SW = 767         # BDs scratch width (relative offsets s = 1..767)
VAL0 = 255       # scratch col of first valid offset (s = 256)
NVALID = 257     # valid offsets s in [256, 512]
NV2 = 258        # band write width (one -1e9 pad col keeps mask intact)
NBUF = 4         # BDs scratch double-buffering depth


def build_kernel():
    nc = bacc.Bacc("TRN2", target_bir_lowering=False, debug=False)

    x_d = nc.dram_tensor("x", [N, DIM], F32, kind="ExternalInput")
    h_d = nc.dram_tensor("h", [M, DIM], F32, kind="ExternalInput")
    wqkv_d = nc.dram_tensor("Wqkv", [DIM, 3 * INNER], F32, kind="ExternalInput")
    wkr_d = nc.dram_tensor("Wkr", [DIM, INNER], F32, kind="ExternalInput")
    r_d = nc.dram_tensor("R", [2 * T, DIM], F32, kind="ExternalInput")
    uu_d = nc.dram_tensor("uu", [128, 1], F32, kind="ExternalInput")
    vv_d = nc.dram_tensor("vv", [128, 1], F32, kind="ExternalInput")
    wout_d = nc.dram_tensor("Wout", [INNER, DIM], F32, kind="ExternalInput")
    out_d = nc.dram_tensor("out", [N, DIM], F32, kind="ExternalOutput")
    bds_d = nc.dram_tensor("bds_scratch", [NBUF, 128, SW], F32)

    with tile.TileContext(nc) as tc, ExitStack() as ctx:
        _body(ctx, tc, x_d, h_d, wqkv_d, wkr_d, r_d, uu_d, vv_d, wout_d,
              out_d, bds_d)

    nc.compile()
    return nc


def _body(ctx, tc, x_d, h_d, wqkv_d, wkr_d, r_d, uu_d, vv_d, wout_d, out_d,
          bds_d):
    nc = tc.nc

    const = ctx.enter_context(tc.tile_pool(name="const", bufs=1))
    persist = ctx.enter_context(tc.tile_pool(name="persist", bufs=1))
    ldpool = ctx.enter_context(tc.tile_pool(name="ld", bufs=4))
    wpool = ctx.enter_context(tc.tile_pool(name="wstream", bufs=3))
    work = ctx.enter_context(tc.tile_pool(name="work", bufs=4))
    ps_big = ctx.enter_context(tc.tile_pool(name="ps_big", bufs=2, space="PSUM"))
    ps_mid = ctx.enter_context(tc.tile_pool(name="ps_mid", bufs=2, space="PSUM"))
    ps_sml = ctx.enter_context(tc.tile_pool(name="ps_sml", bufs=2, space="PSUM"))

    # ---------------- constants ----------------
    ident = const.tile([128, 128], F32, tag="ident", name="ident")
    make_identity(nc, ident)
    ident_h = const.tile([128, 128], F16, tag="identh", name="ident_h")
    make_identity(nc, ident_h)

    # Scratch mask fill: every column outside the per-iteration band write
    # stays NEG; the band write puts NEG in its own pad column.
    neg_sb = const.tile([128, SW], F32, tag="zero", name="neg_sb")
    nc.gpsimd.memset(neg_sb, NEG)

    uu = const.tile([128, 1], F32, tag="uu", name="uu_sb")
    vv = const.tile([128, 1], F32, tag="vv", name="vv_sb")
    nc.sync.dma_start(out=uu, in_=uu_d[:, :])
    nc.sync.dma_start(out=vv, in_=vv_d[:, :])

    zinit = []
    for bi in range(NBUF):
        zi = nc.sync.dma_start(out=bds_d[bi], in_=neg_sb)
        zinit.append(zi)

    # ---------------- weights: gpsimd cast-DMA f32 -> f16 ----------------
    w16 = [persist.tile([128, 3 * INNER], F16, tag=f"w16_{dt}", name=f"w16_{dt}")
           for dt in range(8)]
    for dt in range(8):
        nc.gpsimd.dma_start(out=w16[dt], in_=wqkv_d[dt * 128:(dt + 1) * 128, :])
    wkr16 = [persist.tile([128, INNER], F16, tag=f"wkr16_{dt}", name=f"wkr16_{dt}")
             for dt in range(8)]
    for dt in range(8):
        nc.gpsimd.dma_start(out=wkr16[dt], in_=wkr_d[dt * 128:(dt + 1) * 128, :])

    # ---------------- load + transpose x, h, R ----------------
    # cat token order: [h (0:256) | x (256:512)]
    cat_nat = []
    for tt in range(4):
        t_ = ldpool.tile([128, DIM], F32, tag="xh", name=f"cat_nat{tt}")
        src = h_d if tt < 2 else x_d
        nc.sync.dma_start(out=t_, in_=src[(tt % 2) * 128:(tt % 2) * 128 + 128, :])
        cat_nat.append(t_)

    catT = [persist.tile([128, T], F16, tag=f"catT{dt}", name=f"catT{dt}")
            for dt in range(8)]
    for tt in range(4):
        for dt in range(8):
            tp = ps_sml.tile([128, 128], F32, tag="tp", name=f"tp_cat{tt}_{dt}")
            nc.tensor.transpose(tp, cat_nat[tt][:, dt * 128:(dt + 1) * 128], ident)
            nc.any.tensor_copy(catT[dt][:, tt * 128:(tt + 1) * 128], tp)

    # R rows needed: offsets s=256..511 -> rows 768..1023; s=512 -> row 0
    r_nat = []
    for rt in range(2):
        t_ = ldpool.tile([128, DIM], F32, tag="rn", name=f"r_nat{rt}", bufs=2)
        nc.sync.dma_start(out=t_, in_=r_d[768 + rt * 128:768 + (rt + 1) * 128, :])
        r_nat.append(t_)
    r0 = const.tile([2, DIM], F32, tag="r0", name="r0_sb")
    nc.gpsimd.memset(r0, 0.0)
    nc.sync.dma_start(out=r0[0:1, :], in_=r_d[0:1, :])

    rsubT = [persist.tile([128, NV2], F16, tag=f"rsubT{dt}", name=f"rsubT{dt}")
             for dt in range(8)]
    for rt in range(2):
        for dt in range(8):
            tp = ps_sml.tile([128, 128], F32, tag="tp", name=f"tp_r{rt}_{dt}")
            nc.tensor.transpose(tp, r_nat[rt][:, dt * 128:(dt + 1) * 128], ident)
            nc.any.tensor_copy(rsubT[dt][:, rt * 128:(rt + 1) * 128], tp)
    for dt in range(8):
        tp = ps_sml.tile([128, 2], F32, tag="tp", name=f"tp_r0_{dt}")
        nc.tensor.transpose(tp, r0[:, dt * 128:(dt + 1) * 128], ident[0:2, 0:2])
        nc.any.tensor_copy(rsubT[dt][:, 256:258], tp)

    # ---------------- projections ----------------
    # k_T[ft] = [128 feat, 512 tok]
    kT = [persist.tile([128, T], F16, tag=f"kT{ft}", name=f"kT{ft}")
          for ft in range(8)]
    for ft in range(8):
        pk = ps_mid.tile([128, T], F32, tag="mid", name=f"ps_k{ft}")
        for dt in range(8):
            nc.tensor.matmul(pk, w16[dt][:, INNER + ft * 128:INNER + (ft + 1) * 128],
                             catT[dt], start=(dt == 0), stop=(dt == 7))
        nc.any.tensor_copy(kT[ft], pk)

    # q_T (x tokens only) -> qu_T, qv_T [128 feat, 256 tok]
    quT = [persist.tile([128, N], F16, tag=f"quT{ft}", name=f"quT{ft}")
           for ft in range(8)]
    qvT = [persist.tile([128, N], F16, tag=f"qvT{ft}", name=f"qvT{ft}")
           for ft in range(8)]
    for ft in range(8):
        pq = ps_mid.tile([128, N], F32, tag="mid", name=f"ps_q{ft}")
        for dt in range(8):
            nc.tensor.matmul(pq, w16[dt][:, ft * 128:(ft + 1) * 128],
                             catT[dt][:, M:T], start=(dt == 0), stop=(dt == 7))
        nc.vector.tensor_scalar_add(quT[ft], pq, uu)
        nc.vector.tensor_scalar_add(qvT[ft], pq, vv)

    # RWs_T[ft] = [128 feat, 258 offsets]
    rwsT = [persist.tile([128, NV2], F16, tag=f"rwsT{ft}", name=f"rwsT{ft}")
            for ft in range(8)]
    for ft in range(8):
        pr = ps_mid.tile([128, NV2], F32, tag="mid", name=f"ps_rw{ft}")
        for dt in range(8):
            nc.tensor.matmul(pr, wkr16[dt][:, ft * 128:(ft + 1) * 128],
                             rsubT[dt], start=(dt == 0), stop=(dt == 7))
        nc.any.tensor_copy(rwsT[ft], pr)

    # val[tt] = [128 tok, 1024 feat], two passes of two token tiles each
    val = [persist.tile([128, INNER], F16, tag=f"val{tt}", name=f"val{tt}")
           for tt in range(4)]
    for half in range(2):
        pv = [ps_big.tile([128, INNER], F32, tag="big", name=f"ps_v{half}_{k}")
              for k in range(2)]
        for dt in range(8):
            for k in range(2):
                tt = half * 2 + k
                lhs = catT[dt][:, tt * 128:(tt + 1) * 128]
                for nh in range(2):
                    nc.tensor.matmul(pv[k][:, nh * 512:(nh + 1) * 512],
                                     lhs,
                                     w16[dt][:, 2 * INNER + nh * 512:
                                             2 * INNER + (nh + 1) * 512],
                                     start=(dt == 0), stop=(dt == 7))
        for k in range(2):
            nc.any.tensor_copy(val[half * 2 + k], pv[k])

    # ---------------- attention ----------------
    attn_outT = [persist.tile([128, N], F16, tag=f"aoT{ft}", name=f"aoT{ft}")
                 for ft in range(8)]

    last_read = [None] * NBUF   # WAR chain on the DRAM scratch
    it = 0
    for hh in range(HEADS):
        ft, ro = hh // 2, (hh % 2) * 64
        attnT = [work.tile([128, N], F16, tag="attnT", name=f"attnT{hh}_{jt}",
                           bufs=8) for jt in range(4)]
        for qb in range(2):
            bi = it % NBUF
            qsl = slice(qb * 128, (qb + 1) * 128)

            # BDs = (q+v) @ RWs^T  -> valid band (+ NEG pad col) to scratch
            pb = ps_mid.tile([128, NV2], F32, tag="mid", name=f"ps_b{it}")
            nc.tensor.matmul(pb, qvT[ft][ro:ro + 64, qsl],
                             rwsT[ft][ro:ro + 64, :], start=True, stop=True)
            bsb = work.tile([128, NV2], F32, tag="bsb", name=f"bsb{it}")
            nc.any.tensor_copy(bsb[:, 0:NVALID], pb[:, 0:NVALID])
            nc.vector.memset(bsb[:, NVALID:NV2], NEG)
            w_inst = nc.sync.dma_start(
                out=bds_d[bi][:, VAL0:VAL0 + NV2], in_=bsb)
            add_dep_helper(w_inst.ins, zinit[bi].ins, sync=True,
                           reason="scratch WAW mask-init")
            if last_read[bi] is not None:
                add_dep_helper(w_inst.ins, last_read[bi].ins, sync=True,
                               reason="scratch WAR reuse")

            # A = (q+u) @ k^T; band+mask arrives via the shear read
            pa = ps_mid.tile([128, T], F32, tag="mid", name=f"ps_a{it}")
            nc.tensor.matmul(pa, quT[ft][ro:ro + 64, qsl],
                             kT[ft][ro:ro + 64, :], start=True, stop=True)
            band_sb = work.tile([128, T], F32, tag="band", name=f"band{it}")
            band = bass.AP(bds_d[bi].tensor,
                           bi * 128 * SW + VAL0 - qb * 128,
                           [[SW - 1, 128], [1, T]])
            r_inst = nc.scalar.dma_start(out=band_sb, in_=band)
            add_dep_helper(r_inst.ins, w_inst.ins, sync=True,
                           reason="band RAW on scratch")
            add_dep_helper(r_inst.ins, zinit[bi].ins, sync=True,
                           reason="band RAW on mask-init")
            last_read[bi] = r_inst
            dots = work.tile([128, T], F32, tag="dots", name=f"dots{it}")
            nc.vector.tensor_add(dots, pa, band_sb)

            # exp(+row sums), normalize into fp16 probabilities
            expt = work.tile([128, T], F32, tag="expt", name=f"expt{it}")
            ssum = work.tile([128, 1], F32, tag="ssum", name=f"ssum{it}", bufs=4)
            nc.scalar.activation(expt, dots, AF.Exp, bias=0.0, scale=SCALE,
                                 accum_out=ssum)
            rcp = work.tile([128, 1], F32, tag="rcp", name=f"rcp{it}", bufs=4)
            nc.vector.reciprocal(rcp, ssum)
            expn = work.tile([128, T], F16, tag="expn", name=f"expn{it}")
            nc.vector.tensor_scalar_mul(expn, expt, rcp)

            # transpose attn rows into key-major attnT tiles
            for jt in range(4):
                tp = ps_sml.tile([128, 128], F16, tag="tp", name=f"tp_e{it}_{jt}")
                nc.tensor.transpose(tp, expn[:, jt * 128:(jt + 1) * 128], ident_h)
                nc.any.tensor_copy(attnT[jt][:, qsl], tp)
            it += 1

        # out_T[h] = [64 d, 256 i]
        pav = ps_sml.tile([64, N], F32, tag="tp", name=f"ps_av{hh}")
        for jt in range(4):
            nc.tensor.matmul(pav, val[jt][:, hh * 64:hh * 64 + 64],
                             attnT[jt], start=(jt == 0), stop=(jt == 3))
        nc.any.tensor_copy(attn_outT[ft][ro:ro + 64, :], pav)

    # ---------------- output projection ----------------
    pp = [ps_big.tile([128, DIM], F32, tag="big", name=f"ps_o{tt}")
          for tt in range(2)]
    for itile in range(8):
        wo = wpool.tile([128, DIM], F16, tag="w1024", name=f"wo{itile}")
        nc.gpsimd.dma_start(out=wo, in_=wout_d[itile * 128:(itile + 1) * 128, :])
        for tt in range(2):
            lhs = attn_outT[itile][:, tt * 128:(tt + 1) * 128]
            for nh in range(2):
                nc.tensor.matmul(pp[tt][:, nh * 512:(nh + 1) * 512],
                                 lhs, wo[:, nh * 512:(nh + 1) * 512],
                                 start=(itile == 0), stop=(itile == 7))
    for tt in range(2):
        osb = work.tile([128, DIM], F32, tag="osb", name=f"osb{tt}", bufs=2)
        nc.scalar.copy(osb, pp[tt])
        nc.sync.dma_start(out=out_d[tt * 128:(tt + 1) * 128, :], in_=osb)


_NC_CACHE = {}


def _get_nc():
    if "nc" not in _NC_CACHE:
        _NC_CACHE["nc"] = build_kernel()
    return _NC_CACHE["nc"]


def _run(inputs, trace=False):
    x = np.ascontiguousarray(np.asarray(inputs["x"], dtype=np.float32))
    h = np.ascontiguousarray(np.asarray(inputs["h"], dtype=np.float32))
    wqkv = np.ascontiguousarray(np.asarray(inputs["Wqkv"], dtype=np.float32))
    wkr = np.ascontiguousarray(np.asarray(inputs["Wkr"], dtype=np.float32))
    r = np.ascontiguousarray(np.asarray(inputs["R"], dtype=np.float32))
    u = np.asarray(inputs["u"], dtype=np.float32)
    v = np.asarray(inputs["v"], dtype=np.float32)
    wout = np.ascontiguousarray(np.asarray(inputs["Wout"], dtype=np.float32))
    uu = np.ascontiguousarray(np.tile(u, 2).reshape(128, 1))
    vv = np.ascontiguousarray(np.tile(v, 2).reshape(128, 1))

    nc = _get_nc()
    in_maps = [
        {"x": x[b], "h": h[b], "Wqkv": wqkv, "Wkr": wkr, "R": r,
         "uu": uu, "vv": vv, "Wout": wout}
        for b in range(B)
    ]
    res = bass_utils.run_bass_kernel_spmd(
        nc, in_maps, core_ids=list(range(B)), trace=trace)
    out = np.stack([res.results[b]["out"] for b in range(B)])
    return out.astype(np.float32), res


def kernel(**inputs):
    out, _ = _run(inputs, trace=False)
    return out


# revision 13
# speedup vs baseline: 1.2262x; 1.1300x over previous
# Transformer-XL style relative-position attention on 8 Trainium2 NeuronCores.
#
# Contract: kernel(**inputs) takes the FULL unsharded inputs and returns the
# FULL [8, 256, 1024] output. Internally shards data-parallel over batch:
# core b computes batch element b. No collectives needed.
#
# Math (per batch element):
#   cat = [h; x]                            [512, 1024]
#   q,k,v = split(cat @ Wqkv)               heads=16, dhead=64
#   RW    = R @ Wkr                         [1024, 1024] (relative pos keys)
#   dots  = (q+u) @ k^T + rel_shift((q+v) @ RW_h^T)
#   out   = softmax(dots*8^-1 + causal/mem band mask) @ v @ Wout
#
# Key design points:
#  * The combined mem/autoregressive mask keeps exactly the relative offsets
#    j - i in [0, 256]; in rel-coordinate s = j - i + 256 the valid window is
#    s in [256, 512] (257 values), so only 257 rows of RW are ever needed
#    (R rows 768..1023 and 0, since RW row (s+512)%1024 serves offset s).
#  * rel_shift is a per-row shear. SBUF cannot be addressed diagonally, but
#    DRAM can: write the [128, 258] valid band of BDs = (q+v) @ RWs^T to a
#    DRAM scratch laid out [128, 767] and read it back with the access
#    pattern [[766, 128], [1, 512]] (row stride 767-1) which realizes
#    band[i, j] = BDs[i, j - i + const]. The scratch is pre-filled with the
#    additive mask value -1e9 and the band write puts -1e9 in its pad
#    column, so the band read delivers band+mask in a single tensor.
#  * All matmul operands are fp16 (halves LDWEIGHTS streaming, which
#    dominates the PE pipe); accumulation stays fp32 in PSUM, and the
#    softmax/logit path (dots, exp, row sums) stays fp32.
#  * Weights are cast f32->f16 in-flight by gpsimd (SWDGE) cast-DMAs: no
#    compute-engine time and fully contiguous row reads.
#  * Normalization 1/S is applied per-partition to exp rows before the PE
#    transpose into the key-major layout used by the AV matmul.

import numpy as np

import concourse.bass as bass
import concourse.mybir as mybir
import concourse.tile as tile
from concourse import bacc, bass_utils
from concourse.masks import make_identity
from concourse.tile import add_dep_helper
from contextlib import ExitStack

F32 = mybir.dt.float32
F16 = mybir.dt.float16
AF = mybir.ActivationFunctionType

DIM = 1024
HEADS = 16
DHEAD = 64
B = 8
N = 256          # query tokens (x)
M = 256          # memory tokens (h)
T = M + N        # 512 keys
INNER = HEADS * DHEAD
SCALE = DHEAD ** -0.5
NEG = -1.0e9
SW = 767         # BDs scratch width (relative offsets s = 1..767)
VAL0 = 255       # scratch col of first valid offset (s = 256)
NVALID = 257     # valid offsets s in [256, 512]
NV2 = 258        # band write width (one -1e9 pad col keeps mask intact)
NBUF = 4         # BDs scratch double-buffering depth


def build_kernel():
    nc = bacc.Bacc("TRN2", target_bir_lowering=False, debug=False)

    x_d = nc.dram_tensor("x", [N, DIM], F32, kind="ExternalInput")
    h_d = nc.dram_tensor("h", [M, DIM], F32, kind="ExternalInput")
    wqkv_d = nc.dram_tensor("Wqkv", [DIM, 3 * INNER], F32, kind="ExternalInput")
    wkr_d = nc.dram_tensor("Wkr", [DIM, INNER], F32, kind="ExternalInput")
    r_d = nc.dram_tensor("R", [2 * T, DIM], F32, kind="ExternalInput")
    uu_d = nc.dram_tensor("uu", [128, 1], F32, kind="ExternalInput")
    vv_d = nc.dram_tensor("vv", [128, 1], F32, kind="ExternalInput")
    wout_d = nc.dram_tensor("Wout", [INNER, DIM], F32, kind="ExternalInput")
    out_d = nc.dram_tensor("out", [N, DIM], F32, kind="ExternalOutput")
    bds_d = nc.dram_tensor("bds_scratch", [NBUF, 128, SW], F32)

    with tile.TileContext(nc) as tc, ExitStack() as ctx:
        _body(ctx, tc, x_d, h_d, wqkv_d, wkr_d, r_d, uu_d, vv_d, wout_d,
              out_d, bds_d)

    nc.compile()
    return nc


def _body(ctx, tc, x_d, h_d, wqkv_d, wkr_d, r_d, uu_d, vv_d, wout_d, out_d,
          bds_d):
    nc = tc.nc

    const = ctx.enter_context(tc.tile_pool(name="const", bufs=1))
    persist = ctx.enter_context(tc.tile_pool(name="persist", bufs=1))
    ldpool = ctx.enter_context(tc.tile_pool(name="ld", bufs=4))
    wpool = ctx.enter_context(tc.tile_pool(name="wstream", bufs=3))
    work = ctx.enter_context(tc.tile_pool(name="work", bufs=4))
    ps_big = ctx.enter_context(tc.tile_pool(name="ps_big", bufs=1, space="PSUM"))
    ps_mid = ctx.enter_context(tc.tile_pool(name="ps_mid", bufs=3, space="PSUM"))
    ps_sml = ctx.enter_context(tc.tile_pool(name="ps_sml", bufs=3, space="PSUM"))

    # ---------------- constants ----------------
    ident = const.tile([128, 128], F32, tag="ident", name="ident")
    make_identity(nc, ident)
    ident_h = const.tile([128, 128], F16, tag="identh", name="ident_h")
    make_identity(nc, ident_h)

    # Scratch mask fill: every column outside the per-iteration band write
    # stays NEG; the band write puts NEG in its own pad column.
    neg_sb = const.tile([128, SW], F32, tag="zero", name="neg_sb")
    nc.gpsimd.memset(neg_sb, NEG)

    uu = const.tile([128, 1], F32, tag="uu", name="uu_sb")
    vv = const.tile([128, 1], F32, tag="vv", name="vv_sb")
    nc.sync.dma_start(out=uu, in_=uu_d[:, :])
    nc.sync.dma_start(out=vv, in_=vv_d[:, :])

    zinit = []
    for bi in range(NBUF):
        zi = nc.sync.dma_start(out=bds_d[bi], in_=neg_sb)
        zinit.append(zi)

    # ---------------- weights: gpsimd cast-DMA f32 -> f16 ----------------
    w16 = [persist.tile([128, 3 * INNER], F16, tag=f"w16_{dt}", name=f"w16_{dt}")
           for dt in range(8)]
    for dt in range(8):
        nc.gpsimd.dma_start(out=w16[dt], in_=wqkv_d[dt * 128:(dt + 1) * 128, :])
    wkr16 = [persist.tile([128, INNER], F16, tag=f"wkr16_{dt}", name=f"wkr16_{dt}")
             for dt in range(8)]
    for dt in range(8):
        nc.gpsimd.dma_start(out=wkr16[dt], in_=wkr_d[dt * 128:(dt + 1) * 128, :])
    wo16 = [persist.tile([128, DIM], F16, tag=f"wo16_{dt}", name=f"wo16_{dt}")
            for dt in range(8)]
    for dt in range(8):
        nc.gpsimd.dma_start(out=wo16[dt], in_=wout_d[dt * 128:(dt + 1) * 128, :])

    # ---------------- load + transpose x, h, R ----------------
    # cat token order: [h (0:256) | x (256:512)]
    cat_nat = []
    for tt in range(4):
        t_ = ldpool.tile([128, DIM], F32, tag="xh", name=f"cat_nat{tt}")
        src = h_d if tt < 2 else x_d
        nc.sync.dma_start(out=t_, in_=src[(tt % 2) * 128:(tt % 2) * 128 + 128, :])
        cat_nat.append(t_)

    catT = [persist.tile([128, T], F16, tag=f"catT{dt}", name=f"catT{dt}")
            for dt in range(8)]
    for tt in range(4):
        for dt in range(8):
            tp = ps_sml.tile([128, 128], F32, tag="tp", name=f"tp_cat{tt}_{dt}")
            nc.tensor.transpose(tp, cat_nat[tt][:, dt * 128:(dt + 1) * 128], ident)
            nc.any.tensor_copy(catT[dt][:, tt * 128:(tt + 1) * 128], tp)

    # R rows needed: offsets s=256..511 -> rows 768..1023; s=512 -> row 0
    r_nat = []
    for rt in range(2):
        t_ = ldpool.tile([128, DIM], F32, tag="rn", name=f"r_nat{rt}", bufs=2)
        nc.sync.dma_start(out=t_, in_=r_d[768 + rt * 128:768 + (rt + 1) * 128, :])
        r_nat.append(t_)
    r0 = const.tile([2, DIM], F32, tag="r0", name="r0_sb")
    nc.gpsimd.memset(r0, 0.0)
    nc.sync.dma_start(out=r0[0:1, :], in_=r_d[0:1, :])

    rsubT = [persist.tile([128, NV2], F16, tag=f"rsubT{dt}", name=f"rsubT{dt}")
             for dt in range(8)]
    for rt in range(2):
        for dt in range(8):
            tp = ps_sml.tile([128, 128], F32, tag="tp", name=f"tp_r{rt}_{dt}")
            nc.tensor.transpose(tp, r_nat[rt][:, dt * 128:(dt + 1) * 128], ident)
            nc.any.tensor_copy(rsubT[dt][:, rt * 128:(rt + 1) * 128], tp)
    for dt in range(8):
        tp = ps_sml.tile([128, 2], F32, tag="tp", name=f"tp_r0_{dt}")
        nc.tensor.transpose(tp, r0[:, dt * 128:(dt + 1) * 128], ident[0:2, 0:2])
        nc.any.tensor_copy(rsubT[dt][:, 256:258], tp)

    # ---------------- projections ----------------
    # k_T[ft] = [128 feat, 512 tok]
    kT = [persist.tile([128, T], F16, tag=f"kT{ft}", name=f"kT{ft}")
          for ft in range(8)]
    for ft in range(8):
        pk = ps_mid.tile([128, T], F32, tag="mid", name=f"ps_k{ft}")
        for dt in range(8):
            nc.tensor.matmul(pk, w16[dt][:, INNER + ft * 128:INNER + (ft + 1) * 128],
                             catT[dt], start=(dt == 0), stop=(dt == 7))
        nc.any.tensor_copy(kT[ft], pk)

    # q_T (x tokens only) -> qu_T, qv_T [128 feat, 256 tok]
    quT = [persist.tile([128, N], F16, tag=f"quT{ft}", name=f"quT{ft}")
           for ft in range(8)]
    qvT = [persist.tile([128, N], F16, tag=f"qvT{ft}", name=f"qvT{ft}")
           for ft in range(8)]
    for ft in range(8):
        pq = ps_mid.tile([128, N], F32, tag="mid", name=f"ps_q{ft}")
        for dt in range(8):
            nc.tensor.matmul(pq, w16[dt][:, ft * 128:(ft + 1) * 128],
                             catT[dt][:, M:T], start=(dt == 0), stop=(dt == 7))
        nc.vector.tensor_scalar_add(quT[ft], pq, uu)
        nc.vector.tensor_scalar_add(qvT[ft], pq, vv)

    # RWs_T[ft] = [128 feat, 258 offsets]
    rwsT = [persist.tile([128, NV2], F16, tag=f"rwsT{ft}", name=f"rwsT{ft}")
            for ft in range(8)]
    for ft in range(8):
        pr = ps_mid.tile([128, NV2], F32, tag="mid", name=f"ps_rw{ft}")
        for dt in range(8):
            nc.tensor.matmul(pr, wkr16[dt][:, ft * 128:(ft + 1) * 128],
                             rsubT[dt], start=(dt == 0), stop=(dt == 7))
        nc.any.tensor_copy(rwsT[ft], pr)

    # val[tt] = [128 tok, 1024 feat], two passes of two token tiles each
    val = [persist.tile([128, INNER], F16, tag=f"val{tt}", name=f"val{tt}")
           for tt in range(4)]
    for tt in range(4):
        pv = ps_big.tile([128, INNER], F32, tag="big", name=f"ps_v{tt}")
        for dt in range(8):
            lhs = catT[dt][:, tt * 128:(tt + 1) * 128]
            for nh in range(2):
                nc.tensor.matmul(pv[:, nh * 512:(nh + 1) * 512],
                                 lhs,
                                 w16[dt][:, 2 * INNER + nh * 512:
                                         2 * INNER + (nh + 1) * 512],
                                 start=(dt == 0), stop=(dt == 7))
        nc.any.tensor_copy(val[tt], pv)

    # ---------------- attention ----------------
    attn_outT = [persist.tile([128, N], F16, tag=f"aoT{ft}", name=f"aoT{ft}")
                 for ft in range(8)]

    last_read = [None] * NBUF   # WAR chain on the DRAM scratch
    it = 0
    for hh in range(HEADS):
        ft, ro = hh // 2, (hh % 2) * 64
        attnT = [work.tile([128, N], F16, tag="attnT", name=f"attnT{hh}_{jt}",
                           bufs=8) for jt in range(4)]
        for qb in range(2):
            bi = it % NBUF
            qsl = slice(qb * 128, (qb + 1) * 128)

            # BDs = (q+v) @ RWs^T  -> valid band (+ NEG pad col) to scratch
            pb = ps_mid.tile([128, NV2], F32, tag="mid", name=f"ps_b{it}")
            nc.tensor.matmul(pb, qvT[ft][ro:ro + 64, qsl],
                             rwsT[ft][ro:ro + 64, :], start=True, stop=True)
            bsb = work.tile([128, NV2], F32, tag="bsb", name=f"bsb{it}")
            nc.any.tensor_copy(bsb[:, 0:NVALID], pb[:, 0:NVALID])
            nc.vector.memset(bsb[:, NVALID:NV2], NEG)
            w_inst = nc.sync.dma_start(
                out=bds_d[bi][:, VAL0:VAL0 + NV2], in_=bsb)
            add_dep_helper(w_inst.ins, zinit[bi].ins, sync=True,
                           reason="scratch WAW mask-init")
            if last_read[bi] is not None:
                add_dep_helper(w_inst.ins, last_read[bi].ins, sync=True,
                               reason="scratch WAR reuse")

            # A = (q+u) @ k^T; band+mask arrives via the shear read
            pa = ps_mid.tile([128, T], F32, tag="mid", name=f"ps_a{it}")
            nc.tensor.matmul(pa, quT[ft][ro:ro + 64, qsl],
                             kT[ft][ro:ro + 64, :], start=True, stop=True)
            band_sb = work.tile([128, T], F32, tag="band", name=f"band{it}")
            band = bass.AP(bds_d[bi].tensor,
                           bi * 128 * SW + VAL0 - qb * 128,
                           [[SW - 1, 128], [1, T]])
            r_inst = nc.scalar.dma_start(out=band_sb, in_=band)
            add_dep_helper(r_inst.ins, w_inst.ins, sync=True,
                           reason="band RAW on scratch")
            add_dep_helper(r_inst.ins, zinit[bi].ins, sync=True,
                           reason="band RAW on mask-init")
            last_read[bi] = r_inst
            dots = work.tile([128, T], F32, tag="dots", name=f"dots{it}")
            nc.vector.tensor_add(dots, pa, band_sb)

            # exp(+row sums), normalize into fp16 probabilities
            expt = work.tile([128, T], F32, tag="expt", name=f"expt{it}")
            ssum = work.tile([128, 1], F32, tag="ssum", name=f"ssum{it}", bufs=4)
            nc.scalar.activation(expt, dots, AF.Exp, bias=0.0, scale=SCALE,
                                 accum_out=ssum)
            rcp = work.tile([128, 1], F32, tag="rcp", name=f"rcp{it}", bufs=4)
            nc.vector.reciprocal(rcp, ssum)
            expn = work.tile([128, T], F16, tag="expn", name=f"expn{it}")
            nc.vector.tensor_scalar_mul(expn, expt, rcp)

            # transpose attn rows into key-major attnT tiles
            for jt in range(4):
                tp = ps_sml.tile([128, 128], F16, tag="tp", name=f"tp_e{it}_{jt}")
                nc.tensor.transpose(tp, expn[:, jt * 128:(jt + 1) * 128], ident_h)
                nc.any.tensor_copy(attnT[jt][:, qsl], tp)
            it += 1

        # out_T[h] = [64 d, 256 i]
        pav = ps_sml.tile([64, N], F32, tag="tp", name=f"ps_av{hh}")
        for jt in range(4):
            nc.tensor.matmul(pav, val[jt][:, hh * 64:hh * 64 + 64],
                             attnT[jt], start=(jt == 0), stop=(jt == 3))
        nc.any.tensor_copy(attn_outT[ft][ro:ro + 64, :], pav)

    # ---------------- output projection ----------------
    for tt in range(2):
        pp = ps_big.tile([128, DIM], F32, tag="big", name=f"ps_o{tt}")
        for itile in range(8):
            lhs = attn_outT[itile][:, tt * 128:(tt + 1) * 128]
            for nh in range(2):
                nc.tensor.matmul(pp[:, nh * 512:(nh + 1) * 512],
                                 lhs, wo16[itile][:, nh * 512:(nh + 1) * 512],
                                 start=(itile == 0), stop=(itile == 7))
        osb = work.tile([128, DIM], F32, tag="osb", name=f"osb{tt}", bufs=2)
        nc.scalar.copy(osb, pp)
        nc.sync.dma_start(out=out_d[tt * 128:(tt + 1) * 128, :], in_=osb)


_NC_CACHE = {}


def _get_nc():
    if "nc" not in _NC_CACHE:
        _NC_CACHE["nc"] = build_kernel()
    return _NC_CACHE["nc"]


def _run(inputs, trace=False):
    x = np.ascontiguousarray(np.asarray(inputs["x"], dtype=np.float32))
    h = np.ascontiguousarray(np.asarray(inputs["h"], dtype=np.float32))
    wqkv = np.ascontiguousarray(np.asarray(inputs["Wqkv"], dtype=np.float32))
    wkr = np.ascontiguousarray(np.asarray(inputs["Wkr"], dtype=np.float32))
    r = np.ascontiguousarray(np.asarray(inputs["R"], dtype=np.float32))
    u = np.asarray(inputs["u"], dtype=np.float32)
    v = np.asarray(inputs["v"], dtype=np.float32)
    wout = np.ascontiguousarray(np.asarray(inputs["Wout"], dtype=np.float32))
    uu = np.ascontiguousarray(np.tile(u, 2).reshape(128, 1))
    vv = np.ascontiguousarray(np.tile(v, 2).reshape(128, 1))

    nc = _get_nc()
    in_maps = [
        {"x": x[b], "h": h[b], "Wqkv": wqkv, "Wkr": wkr, "R": r,
         "uu": uu, "vv": vv, "Wout": wout}
        for b in range(B)
    ]
    res = bass_utils.run_bass_kernel_spmd(
        nc, in_maps, core_ids=list(range(B)), trace=trace)
    out = np.stack([res.results[b]["out"] for b in range(B)])
    return out.astype(np.float32), res


def kernel(**inputs):
    out, _ = _run(inputs, trace=False)
    return out


# revision 15
# speedup vs baseline: 1.2802x; 1.0440x over previous
# Transformer-XL style relative-position attention on 8 Trainium2 NeuronCores.
#
# Contract: kernel(**inputs) takes the FULL unsharded inputs and returns the
# FULL [8, 256, 1024] output. Internally shards data-parallel over batch:
# core b computes batch element b. No collectives needed.
#
# Math (per batch element):
#   cat = [h; x]                            [512, 1024]
#   q,k,v = split(cat @ Wqkv)               heads=16, dhead=64
#   RW    = R @ Wkr                         [1024, 1024] (relative pos keys)
#   dots  = (q+u) @ k^T + rel_shift((q+v) @ RW_h^T)
#   out   = softmax(dots*8^-1 + causal/mem band mask) @ v @ Wout
#
# Key design points:
#  * The combined mem/autoregressive mask keeps exactly the relative offsets
#    j - i in [0, 256]; in rel-coordinate s = j - i + 256 the valid window is
#    s in [256, 512] (257 values), so only 257 rows of RW are ever needed
#    (R rows 768..1023 and 0, since RW row (s+512)%1024 serves offset s).
#  * rel_shift is a per-row shear. SBUF cannot be addressed diagonally, but
#    DRAM can: write the [128, 258] valid band of BDs = (q+v) @ RWs^T to a
#    DRAM scratch laid out [128, 767] and read it back with the access
#    pattern [[766, 128], [1, 512]] (row stride 767-1) which realizes
#    band[i, j] = BDs[i, j - i + const]. The scratch is pre-filled with the
#    additive mask value -1e9 and the band write puts -1e9 in its pad
#    column, so the band read delivers band+mask in a single tensor.
#  * All matmul operands are fp16 (halves LDWEIGHTS streaming, which
#    dominates the PE pipe); accumulation stays fp32 in PSUM, and the
#    softmax/logit path (dots, exp, row sums) stays fp32.
#  * Weights are cast f32->f16 in-flight by gpsimd (SWDGE) cast-DMAs: no
#    compute-engine time and fully contiguous row reads.
#  * Normalization 1/S is applied per-partition to exp rows before the PE
#    transpose into the key-major layout used by the AV matmul.

import numpy as np

import concourse.bass as bass
import concourse.mybir as mybir
import concourse.tile as tile
from concourse import bacc, bass_utils
from concourse.masks import make_identity
from concourse.tile import add_dep_helper
from contextlib import ExitStack

F32 = mybir.dt.float32
F16 = mybir.dt.float16
AF = mybir.ActivationFunctionType

DIM = 1024
HEADS = 16
DHEAD = 64
B = 8
N = 256          # query tokens (x)
M = 256          # memory tokens (h)
T = M + N        # 512 keys
INNER = HEADS * DHEAD
SCALE = DHEAD ** -0.5
NEG = -1.0e9
SW = 767         # BDs scratch width (relative offsets s = 1..767)
VAL0 = 255       # scratch col of first valid offset (s = 256)
NVALID = 257     # valid offsets s in [256, 512]
NV2 = 258        # band write width (one -1e9 pad col keeps mask intact)
WIN = 384        # per-query-block live key window (3 of 4 key tiles)
NBUF = 4         # BDs scratch double-buffering depth


def build_kernel():
    nc = bacc.Bacc("TRN2", target_bir_lowering=False, debug=False)

    x_d = nc.dram_tensor("x", [N, DIM], F32, kind="ExternalInput")
    h_d = nc.dram_tensor("h", [M, DIM], F32, kind="ExternalInput")
    wqkv_d = nc.dram_tensor("Wqkv", [DIM, 3 * INNER], F32, kind="ExternalInput")
    wkr_d = nc.dram_tensor("Wkr", [DIM, INNER], F32, kind="ExternalInput")
    r_d = nc.dram_tensor("R", [2 * T, DIM], F32, kind="ExternalInput")
    uu_d = nc.dram_tensor("uu", [128, 1], F32, kind="ExternalInput")
    vv_d = nc.dram_tensor("vv", [128, 1], F32, kind="ExternalInput")
    wout_d = nc.dram_tensor("Wout", [INNER, DIM], F32, kind="ExternalInput")
    out_d = nc.dram_tensor("out", [N, DIM], F32, kind="ExternalOutput")
    bds_d = nc.dram_tensor("bds_scratch", [NBUF, 128, SW], F32)

    with tile.TileContext(nc) as tc, ExitStack() as ctx:
        _body(ctx, tc, x_d, h_d, wqkv_d, wkr_d, r_d, uu_d, vv_d, wout_d,
              out_d, bds_d)

    nc.compile()
    return nc


def _body(ctx, tc, x_d, h_d, wqkv_d, wkr_d, r_d, uu_d, vv_d, wout_d, out_d,
          bds_d):
    nc = tc.nc

    const = ctx.enter_context(tc.tile_pool(name="const", bufs=1))
    persist = ctx.enter_context(tc.tile_pool(name="persist", bufs=1))
    ldpool = ctx.enter_context(tc.tile_pool(name="ld", bufs=4))
    wpool = ctx.enter_context(tc.tile_pool(name="wstream", bufs=3))
    work = ctx.enter_context(tc.tile_pool(name="work", bufs=4))
    ps_big = ctx.enter_context(tc.tile_pool(name="ps_big", bufs=1, space="PSUM"))
    ps_mid = ctx.enter_context(tc.tile_pool(name="ps_mid", bufs=3, space="PSUM"))
    ps_sml = ctx.enter_context(tc.tile_pool(name="ps_sml", bufs=3, space="PSUM"))

    # ---------------- constants ----------------
    ident = const.tile([128, 128], F32, tag="ident", name="ident")
    make_identity(nc, ident)
    ident_h = const.tile([128, 128], F16, tag="identh", name="ident_h")
    make_identity(nc, ident_h)

    # Scratch mask fill: every column outside the per-iteration band write
    # stays NEG; the band write puts NEG in its own pad column.
    neg_sb = const.tile([128, SW], F32, tag="zero", name="neg_sb")
    nc.gpsimd.memset(neg_sb, NEG)

    uu = const.tile([128, 1], F32, tag="uu", name="uu_sb")
    vv = const.tile([128, 1], F32, tag="vv", name="vv_sb")
    nc.sync.dma_start(out=uu, in_=uu_d[:, :])
    nc.sync.dma_start(out=vv, in_=vv_d[:, :])

    zinit = []
    for bi in range(NBUF):
        zi = nc.sync.dma_start(out=bds_d[bi], in_=neg_sb)
        zinit.append(zi)

    # ---------------- weights: gpsimd cast-DMA f32 -> f16 ----------------
    w16 = [persist.tile([128, 3 * INNER], F16, tag=f"w16_{dt}", name=f"w16_{dt}")
           for dt in range(8)]
    for dt in range(8):
        nc.gpsimd.dma_start(out=w16[dt], in_=wqkv_d[dt * 128:(dt + 1) * 128, :])
    wkr16 = [persist.tile([128, INNER], F16, tag=f"wkr16_{dt}", name=f"wkr16_{dt}")
             for dt in range(8)]
    for dt in range(8):
        nc.gpsimd.dma_start(out=wkr16[dt], in_=wkr_d[dt * 128:(dt + 1) * 128, :])
    wo16 = [persist.tile([128, DIM], F16, tag=f"wo16_{dt}", name=f"wo16_{dt}")
            for dt in range(8)]
    for dt in range(8):
        nc.gpsimd.dma_start(out=wo16[dt], in_=wout_d[dt * 128:(dt + 1) * 128, :])

    # ---------------- load + transpose x, h, R ----------------
    # cat token order: [h (0:256) | x (256:512)]; loads cast f32->f16 in the
    # gpsimd DMA, then SBUF->SBUF DMA-transposes build the feature-major
    # layouts with no PE or PSUM involvement.
    cat16 = []
    for tt in range(4):
        t_ = ldpool.tile([128, DIM], F16, tag="xh", name=f"cat16_{tt}")
        src = h_d if tt < 2 else x_d
        nc.gpsimd.dma_start(out=t_, in_=src[(tt % 2) * 128:(tt % 2) * 128 + 128, :])
        cat16.append(t_)

    catT = [persist.tile([128, T], F16, tag=f"catT{dt}", name=f"catT{dt}")
            for dt in range(8)]
    for tt in range(4):
        for dt in range(8):
            tp = ps_sml.tile([128, 128], F16, tag="tp", name=f"tp_cat{tt}_{dt}")
            nc.tensor.transpose(tp, cat16[tt][:, dt * 128:(dt + 1) * 128],
                                ident_h)
            nc.any.tensor_copy(catT[dt][:, tt * 128:(tt + 1) * 128], tp)

    # R rows needed: offsets s=256..511 -> rows 768..1023; s=512 -> row 0
    r16 = []
    for rt in range(2):
        t_ = ldpool.tile([128, DIM], F16, tag="rn", name=f"r16_{rt}", bufs=2)
        nc.gpsimd.dma_start(out=t_, in_=r_d[768 + rt * 128:768 + (rt + 1) * 128, :])
        r16.append(t_)
    r0 = const.tile([2, DIM], F32, tag="r0", name="r0_sb")
    nc.gpsimd.memset(r0, 0.0)
    nc.sync.dma_start(out=r0[0:1, :], in_=r_d[0:1, :])

    rsubT = [persist.tile([128, NV2], F16, tag=f"rsubT{dt}", name=f"rsubT{dt}")
             for dt in range(8)]
    for rt in range(2):
        for dt in range(8):
            tp = ps_sml.tile([128, 128], F16, tag="tp", name=f"tp_r{rt}_{dt}")
            nc.tensor.transpose(tp, r16[rt][:, dt * 128:(dt + 1) * 128],
                                ident_h)
            nc.any.tensor_copy(rsubT[dt][:, rt * 128:(rt + 1) * 128], tp)
    for dt in range(8):
        tp = ps_sml.tile([128, 2], F32, tag="tp", name=f"tp_r0_{dt}")
        nc.tensor.transpose(tp, r0[:, dt * 128:(dt + 1) * 128], ident[0:2, 0:2])
        nc.any.tensor_copy(rsubT[dt][:, 256:258], tp)

    # ---------------- projections ----------------
    # k_T[ft] = [128 feat, 512 tok]
    kT = [persist.tile([128, T], F16, tag=f"kT{ft}", name=f"kT{ft}")
          for ft in range(8)]
    for ft in range(8):
        pk = ps_mid.tile([128, T], F32, tag="mid", name=f"ps_k{ft}")
        for dt in range(8):
            nc.tensor.matmul(pk, w16[dt][:, INNER + ft * 128:INNER + (ft + 1) * 128],
                             catT[dt], start=(dt == 0), stop=(dt == 7))
        nc.any.tensor_copy(kT[ft], pk)

    # q_T (x tokens only) -> qu_T, qv_T [128 feat, 256 tok]
    quT = [persist.tile([128, N], F16, tag=f"quT{ft}", name=f"quT{ft}")
           for ft in range(8)]
    qvT = [persist.tile([128, N], F16, tag=f"qvT{ft}", name=f"qvT{ft}")
           for ft in range(8)]
    for ft in range(8):
        pq = ps_mid.tile([128, N], F32, tag="mid", name=f"ps_q{ft}")
        for dt in range(8):
            nc.tensor.matmul(pq, w16[dt][:, ft * 128:(ft + 1) * 128],
                             catT[dt][:, M:T], start=(dt == 0), stop=(dt == 7))
        nc.vector.tensor_scalar_add(quT[ft], pq, uu)
        nc.vector.tensor_scalar_add(qvT[ft], pq, vv)

    # RWs_T[ft] = [128 feat, 258 offsets]
    rwsT = [persist.tile([128, NV2], F16, tag=f"rwsT{ft}", name=f"rwsT{ft}")
            for ft in range(8)]
    for ft in range(8):
        pr = ps_mid.tile([128, NV2], F32, tag="mid", name=f"ps_rw{ft}")
        for dt in range(8):
            nc.tensor.matmul(pr, wkr16[dt][:, ft * 128:(ft + 1) * 128],
                             rsubT[dt], start=(dt == 0), stop=(dt == 7))
        nc.any.tensor_copy(rwsT[ft], pr)

    # val[tt] = [128 tok, 1024 feat], two passes of two token tiles each
    val = [persist.tile([128, INNER], F16, tag=f"val{tt}", name=f"val{tt}")
           for tt in range(4)]
    for tt in range(4):
        pv = ps_big.tile([128, INNER], F32, tag="big", name=f"ps_v{tt}")
        for dt in range(8):
            lhs = catT[dt][:, tt * 128:(tt + 1) * 128]
            for nh in range(2):
                nc.tensor.matmul(pv[:, nh * 512:(nh + 1) * 512],
                                 lhs,
                                 w16[dt][:, 2 * INNER + nh * 512:
                                         2 * INNER + (nh + 1) * 512],
                                 start=(dt == 0), stop=(dt == 7))
        nc.any.tensor_copy(val[tt], pv)

    # ---------------- attention ----------------
    attn_outT = [persist.tile([128, N], F16, tag=f"aoT{ft}", name=f"aoT{ft}")
                 for ft in range(8)]

    last_read = [None] * NBUF   # WAR chain on the DRAM scratch
    it = 0
    for hh in range(HEADS):
        ft, ro = hh // 2, (hh % 2) * 64
        attnT = [work.tile([128, N], F16, tag="attnT", name=f"attnT{hh}_{jt}",
                           bufs=8) for jt in range(4)]
        # halves never written by the windowed transposes must be zero
        nc.vector.memset(attnT[0][:, 128:256], 0.0)
        nc.vector.memset(attnT[3][:, 0:128], 0.0)
        for qb in range(2):
            bi = it % NBUF
            qsl = slice(qb * 128, (qb + 1) * 128)

            # BDs = (q+v) @ RWs^T  -> valid band (+ NEG pad col) to scratch
            pb = ps_mid.tile([128, NV2], F32, tag="mid", name=f"ps_b{it}")
            nc.tensor.matmul(pb, qvT[ft][ro:ro + 64, qsl],
                             rwsT[ft][ro:ro + 64, :], start=True, stop=True)
            bsb = work.tile([128, NV2], F32, tag="bsb", name=f"bsb{it}")
            nc.any.tensor_copy(bsb[:, 0:NVALID], pb[:, 0:NVALID])
            nc.vector.memset(bsb[:, NVALID:NV2], NEG)
            w_inst = nc.sync.dma_start(
                out=bds_d[bi][:, VAL0:VAL0 + NV2], in_=bsb)
            add_dep_helper(w_inst.ins, zinit[bi].ins, sync=True,
                           reason="scratch WAW mask-init")
            if last_read[bi] is not None:
                add_dep_helper(w_inst.ins, last_read[bi].ins, sync=True,
                               reason="scratch WAR reuse")

            # A = (q+u) @ k^T over the live 384-key window
            pa = ps_mid.tile([128, WIN], F32, tag="mid", name=f"ps_a{it}")
            nc.tensor.matmul(pa, quT[ft][ro:ro + 64, qsl],
                             kT[ft][ro:ro + 64, qb * 128:qb * 128 + WIN],
                             start=True, stop=True)
            band_sb = work.tile([128, WIN], F32, tag="band", name=f"band{it}")
            band = bass.AP(bds_d[bi].tensor,
                           bi * 128 * SW + VAL0,
                           [[SW - 1, 128], [1, WIN]])
            r_inst = nc.scalar.dma_start(out=band_sb, in_=band)
            add_dep_helper(r_inst.ins, w_inst.ins, sync=True,
                           reason="band RAW on scratch")
            add_dep_helper(r_inst.ins, zinit[bi].ins, sync=True,
                           reason="band RAW on mask-init")
            last_read[bi] = r_inst
            dots = work.tile([128, WIN], F32, tag="dots", name=f"dots{it}")
            nc.vector.tensor_add(dots, pa, band_sb)

            # exp(+row sums), normalize into fp16 probabilities
            expt = work.tile([128, WIN], F32, tag="expt", name=f"expt{it}")
            ssum = work.tile([128, 1], F32, tag="ssum", name=f"ssum{it}", bufs=4)
            nc.scalar.activation(expt, dots, AF.Exp, bias=0.0, scale=SCALE,
                                 accum_out=ssum)
            rcp = work.tile([128, 1], F32, tag="rcp", name=f"rcp{it}", bufs=4)
            nc.vector.reciprocal(rcp, ssum)
            expn = work.tile([128, WIN], F16, tag="expn", name=f"expn{it}")
            nc.vector.tensor_scalar_mul(expn, expt, rcp)

            # transpose attn rows into key-major attnT tiles (3 live tiles)
            for w in range(3):
                jt = qb + w
                tp = ps_sml.tile([128, 128], F16, tag="tp", name=f"tp_e{it}_{w}")
                nc.tensor.transpose(tp, expn[:, w * 128:(w + 1) * 128], ident_h)
                nc.any.tensor_copy(attnT[jt][:, qsl], tp)
            it += 1

        # out_T[h] = [64 d, 256 i]
        pav = ps_sml.tile([64, N], F32, tag="tp", name=f"ps_av{hh}")
        for jt in range(4):
            nc.tensor.matmul(pav, val[jt][:, hh * 64:hh * 64 + 64],
                             attnT[jt], start=(jt == 0), stop=(jt == 3))
        nc.any.tensor_copy(attn_outT[ft][ro:ro + 64, :], pav)

    # ---------------- output projection ----------------
    for tt in range(2):
        pp = ps_big.tile([128, DIM], F32, tag="big", name=f"ps_o{tt}")
        for itile in range(8):
            lhs = attn_outT[itile][:, tt * 128:(tt + 1) * 128]
            for nh in range(2):
                nc.tensor.matmul(pp[:, nh * 512:(nh + 1) * 512],
                                 lhs, wo16[itile][:, nh * 512:(nh + 1) * 512],
                                 start=(itile == 0), stop=(itile == 7))
        osb = work.tile([128, DIM], F32, tag="osb", name=f"osb{tt}", bufs=2)
        nc.scalar.copy(osb, pp)
        nc.sync.dma_start(out=out_d[tt * 128:(tt + 1) * 128, :], in_=osb)


_NC_CACHE = {}


def _get_nc():
    if "nc" not in _NC_CACHE:
        _NC_CACHE["nc"] = build_kernel()
    return _NC_CACHE["nc"]


def _run(inputs, trace=False):
    x = np.ascontiguousarray(np.asarray(inputs["x"], dtype=np.float32))
    h = np.ascontiguousarray(np.asarray(inputs["h"], dtype=np.float32))
    wqkv = np.ascontiguousarray(np.asarray(inputs["Wqkv"], dtype=np.float32))
    wkr = np.ascontiguousarray(np.asarray(inputs["Wkr"], dtype=np.float32))
    r = np.ascontiguousarray(np.asarray(inputs["R"], dtype=np.float32))
    u = np.asarray(inputs["u"], dtype=np.float32)
    v = np.asarray(inputs["v"], dtype=np.float32)
    wout = np.ascontiguousarray(np.asarray(inputs["Wout"], dtype=np.float32))
    uu = np.ascontiguousarray(np.tile(u, 2).reshape(128, 1))
    vv = np.ascontiguousarray(np.tile(v, 2).reshape(128, 1))

    nc = _get_nc()
    in_maps = [
        {"x": x[b], "h": h[b], "Wqkv": wqkv, "Wkr": wkr, "R": r,
         "uu": uu, "vv": vv, "Wout": wout}
        for b in range(B)
    ]
    res = bass_utils.run_bass_kernel_spmd(
        nc, in_maps, core_ids=list(range(B)), trace=trace)
    out = np.stack([res.results[b]["out"] for b in range(B)])
    return out.astype(np.float32), res


def kernel(**inputs):
    out, _ = _run(inputs, trace=False)
    return out
